# revision 1
# baseline (speedup 1.0000x reference)
"""2D DCT-II (separable) kernel for Trainium2, data-parallel over 8 NeuronCores.

Problem: img [128, 1, 512, 512] f32 -> out [128, 1, 512, 512] f32 with
    out[b,0] = scale * (Cp @ img[b,0] @ Cq^T),  Cp == Cq, scale rank-1 folded in:
    C'[k,j] = s_k * cos(pi*(2j+1)*k/1024),  s_k = sqrt(2/512) * (1/sqrt2 if k==0)
    out[b] = C' @ A @ C'^T

Scheme (all bf16; rel-err budget 2e-2 >> bf16's ~4e-3):
The DCT reflection symmetry C'[k, 511-j] = (-1)^k C'[k, j] folds BOTH
contractions from 512 to 256 ("quadrant folding"), halving PE work vs the
unfolded separable transform. The host stages rows AND columns in the
self-pairing order [0..127, 255..128, 511..384, 256..383] (pure permutation)
so every fold is partition/column-aligned, and stores the basis in the same
permuted contraction order (contraction order is free).

    rowE/rowO[m', c] = A[m', c] +/- A[pair(m'), c]        (DVE add / Pool sub)
    stage1 (per row-parity r, basis B_r):                  16 matmuls x 256
        psL[n', p'] = sum_m' rowX[m', n']      B_r[m', p']   (left col half)
        psR[n', p'] = sum_m' rowX[m', n'+256]  B_r[m', p']   (right col half)
    col fold fused into the psum->sbuf copy (DVE):
        dt_re = psL + psR, dt_ro = psL - psR               (bf16)
    stage2 (per quadrant rc, basis B_c):                   16 matmuls x 256
        Y_rc[p', q'] = sum_n' dt_rc[n', p'] B_c[n', q']
    out[2p'+r, 2q'+c] = Y_rc[p', q']                       (host un-permutes)

Per image: 32 bf16 matmuls x 256 free columns (8192 PE cycles at 2.4 GHz vs
12288 fp32r cycles for the single-fold baseline). bf16 stationary loads use
the fast-weight-load path so LDWEIGHTS hides under the matmuls. Stage 2 runs
one image behind stage 1 (software pipeline) so the fused fold-copies are
never on the PE critical path. bf16 I/O halves DMA to 1 MB/image.
"""

import sys
import numpy as np
import ml_dtypes

for _p in ("/opt/trn_rl_repo", "/opt/pypackages"):
    if _p not in sys.path:
        sys.path.append(_p)

import concourse.tile as tile  # noqa: E402
from concourse import bacc, mybir  # noqa: E402
from concourse.bass_utils import run_bass_kernel_spmd  # noqa: E402

N_CORES = 8
B_FULL = 128
S = 512
H = 256
BPC = B_FULL // N_CORES  # images per core

BF16 = mybir.dt.bfloat16

# Stored index -> original index, self-similar fold order (rows and columns).
PERM = np.concatenate(
    [
        np.arange(0, 128),
        np.arange(255, 127, -1),
        np.arange(511, 383, -1),
        np.arange(256, 384),
    ]
)
PERM256 = PERM[:256]


def _basis_np():
    """B_e/B_o [256 stored-contraction, 256 out] in the stored (permuted) order."""
    j = np.arange(S, dtype=np.float64)
    k = np.arange(S, dtype=np.float64)
    c = np.cos(np.pi * (2.0 * j[None, :] + 1.0) * k[:, None] / (2.0 * S))
    s = np.full(S, np.sqrt(2.0 / S))
    s[0] /= np.sqrt(2.0)
    C = c * s[:, None]  # C'[k, j]
    ET = C[0::2, :][:, PERM256].T.copy()  # [256 stored m', 256 p']
    OT = C[1::2, :][:, PERM256].T.copy()

    def to_tiles(M):  # [256, 256] -> [128, 2, 256]
        return np.ascontiguousarray(
            M.reshape(2, 128, 256).transpose(1, 0, 2)
        ).astype(ml_dtypes.bfloat16)

    return to_tiles(ET), to_tiles(OT)


def _build():
    nc = bacc.Bacc("TRN2", target_bir_lowering=False, debug=False)
    in_d = nc.dram_tensor("inp", [BPC, 128, 4, S], BF16, kind="ExternalInput").ap()
    et_d = nc.dram_tensor("et", [128, 2, H], BF16, kind="ExternalInput").ap()
    ot_d = nc.dram_tensor("ot", [128, 2, H], BF16, kind="ExternalInput").ap()
    out_d = nc.dram_tensor(
        "out", [BPC, 128, 2, 2, 2, H], BF16, kind="ExternalOutput"
    ).ap()

    with tile.TileContext(nc) as tc:
        with (
            tc.tile_pool(name="const", bufs=1) as cpool,
            tc.tile_pool(name="a", bufs=4) as apool,
            tc.tile_pool(name="a0", bufs=1) as a0pool,
            tc.tile_pool(name="row", bufs=4) as rpool,
            tc.tile_pool(name="dt", bufs=8) as dtpool,
            tc.tile_pool(name="slp", bufs=4) as slpool,
            tc.tile_pool(name="st", bufs=3) as stpool,
            tc.tile_pool(name="ps1", bufs=4, space="PSUM") as ps1pool,
            tc.tile_pool(name="ps2", bufs=2, space="PSUM") as ps2pool,
        ):
            et_sb = cpool.tile([128, 2, H], BF16)
            ot_sb = cpool.tile([128, 2, H], BF16)
            bas = {"e": et_sb, "o": ot_sb}
            # PE warm-up on a never-written tile (values irrelevant, results
            # unread): no data dependency, so the PE is busy from ucode-load
            # time and the HAM clock-gate is at 2.4 GHz for the real matmuls.
            junk = cpool.tile([128, 2, H], BF16)
            nc.gpsimd.memset(junk[:], 0)

            def emit_load(i):
                a = apool.tile([128, 4, S], BF16, tag="a", name=f"a_{i}")
                nc.sync.dma_start(a[:], in_d[i])
                return a

            def emit_folds(i, a):
                """rowE = top + bottom (DVE, 2x add), rowO = top - bottom (Pool)."""
                rowE = rpool.tile([128, 2, S], BF16, tag="row", name=f"re_{i}")
                rowO = rpool.tile([128, 2, S], BF16, tag="row", name=f"ro_{i}")
                nc.vector.tensor_add(rowE[:], a[:, 0:2, :], a[:, 2:4, :])
                nc.gpsimd.tensor_sub(rowO[:], a[:, 0:2, :], a[:, 2:4, :])
                return rowE, rowO

            wu = ps2pool.tile([128, 2, 2, H], mybir.dt.float32, tag="ps2", name="warmup")
            for k in range(11):
                nc.tensor.matmul(
                    wu[:, k % 2, 0, :],
                    junk[:, 0, 0:128],
                    junk[:, k % 2, :],
                    start=True,
                    stop=True,
                )
            # ---- fast-path startup: image 0 loads as two fold-pair halves so
            # its folds (and first matmuls) start as early as possible; the
            # basis rides the idle Scalar DMA queue in parallel.
            a0A = a0pool.tile([128, 2, S], BF16, name="a0A")
            a0B = a0pool.tile([128, 2, S], BF16, name="a0B")
            nc.sync.dma_start(a0A[:], in_d[0, :, 0:3:2, :])  # groups 0, 2
            nc.scalar.dma_start(et_sb[:], et_d)
            nc.scalar.dma_start(ot_sb[:], ot_d)
            nc.sync.dma_start(a0B[:], in_d[0, :, 1:4:2, :])  # groups 1, 3
            a1 = emit_load(1)
            # image-0 folds, split per row-tile so t=0 follows the first DMA
            rowE0 = rpool.tile([128, 2, S], BF16, tag="row", name="re_0")
            rowO0 = rpool.tile([128, 2, S], BF16, tag="row", name="ro_0")
            nc.vector.tensor_add(rowE0[:, 0, :], a0A[:, 0, :], a0A[:, 1, :])
            nc.gpsimd.tensor_sub(rowO0[:, 0, :], a0A[:, 0, :], a0A[:, 1, :])
            nc.vector.tensor_add(rowE0[:, 1, :], a0B[:, 0, :], a0B[:, 1, :])
            nc.gpsimd.tensor_sub(rowO0[:, 1, :], a0B[:, 0, :], a0B[:, 1, :])
            pend = {1: a1}
            cur_folds = (rowE0, rowO0)
            prev_dt = None

            for i in range(BPC + 1):
                nxt_dt = None
                if i < BPC:
                    rowX = {"e": cur_folds[0], "o": cur_folds[1]}
                    # ---- stage 1: psL/psR per row-parity, [128, ns(2), 256]
                    ps = {}
                    for r in ("e", "o"):
                        src, b = rowX[r], bas[r]
                        pl = ps1pool.tile(
                            [128, 2, H], mybir.dt.float32, tag="ps1", name=f"pl_{i}_{r}"
                        )
                        pr = ps1pool.tile(
                            [128, 2, H], mybir.dt.float32, tag="ps1", name=f"pr_{i}_{r}"
                        )
                        for half, pst in ((0, pl), (1, pr)):
                            for ns in range(2):
                                for t in range(2):
                                    o = half * H + ns * 128
                                    nc.tensor.matmul(
                                        pst[:, ns, :],
                                        src[:, t, o : o + 128],
                                        b[:, t, :],
                                        start=(t == 0),
                                        stop=(t == 1),
                                    )
                        ps[r] = (pl, pr)
                    # prefetch + next image's row folds
                    if i + 2 < BPC:
                        pend[i + 2] = emit_load(i + 2)
                    if i + 1 < BPC:
                        cur_folds = emit_folds(i + 1, pend.pop(i + 1))
                    # ---- fused col-fold copies: dt_re = psL+psR, dt_ro = psL-psR
                    # (dual-PSUM tensor_tensor is illegal, so stage psL to SBUF
                    # f32 on ACT, then DVE combines SBUF + PSUM.)
                    nxt_dt = {}
                    for r in ("e", "o"):
                        pl, pr = ps[r]
                        sl = slpool.tile(
                            [128, 2, H], mybir.dt.float32, tag="sl", name=f"sl_{i}_{r}"
                        )
                        nc.scalar.copy(sl[:], pl[:])
                        da = dtpool.tile([128, 2, H], BF16, tag="dt", name=f"da_{i}_{r}")
                        ds = dtpool.tile([128, 2, H], BF16, tag="dt", name=f"ds_{i}_{r}")
                        nc.vector.tensor_add(da[:], sl[:], pr[:])
                        nc.vector.tensor_sub(ds[:], sl[:], pr[:])
                        nxt_dt[r] = (da, ds)

                if i >= 1:
                    # ---- stage 2 for image i-1: Y_rc[p', q']
                    j = i - 1
                    st = stpool.tile([128, 2, 2, 2, H], BF16, tag="st", name=f"st_{j}")
                    for pi, r in enumerate(("e", "o")):
                        da, ds = prev_dt[r]
                        p2 = ps2pool.tile(
                            [128, 2, 2, H], mybir.dt.float32, tag="ps2", name=f"p2_{j}_{r}"
                        )
                        for qi, (dtq, c) in enumerate(((da, "e"), (ds, "o"))):
                            for psl in range(2):
                                for t2 in range(2):
                                    nc.tensor.matmul(
                                        p2[:, qi, psl, :],
                                        dtq[:, t2, psl * 128 : (psl + 1) * 128],
                                        bas[c][:, t2, :],
                                        start=(t2 == 0),
                                        stop=(t2 == 1),
                                    )
                            if j == BPC - 1:
                                # tail drain: copy each quarter right after its
                                # matmuls (ACT/DVE split), DMA it immediately
                                cp = nc.scalar.copy if pi == 0 else nc.vector.tensor_copy
                                cp(st[:, pi, qi], p2[:, qi])
                                eng = nc.scalar if pi == 0 else nc.sync
                                eng.dma_start(out_d[j, :, pi, qi], st[:, pi, qi])
                        if j < BPC - 1:
                            nc.scalar.copy(st[:, pi], p2[:])
                    if j < BPC - 1:
                        nc.sync.dma_start(out_d[j], st[:])
                prev_dt = nxt_dt
    nc.compile()
    return nc


_NC_CACHE = None


def _get_nc():
    global _NC_CACHE
    if _NC_CACHE is None:
        _NC_CACHE = _build()
    return _NC_CACHE


def run_sharded(img: np.ndarray, **spmd_kwargs):
    """img [128, 1, 512, 512] f32 -> (out [128, 1, 512, 512] f32, results)."""
    img = np.asarray(img, dtype=np.float32).reshape(B_FULL, S, S)
    # host staging: permute rows+cols into fold order, tile rows into 4 groups
    x = img[:, PERM, :][:, :, PERM]
    xt = np.ascontiguousarray(
        x.reshape(B_FULL, 4, 128, S).transpose(0, 2, 1, 3)
    ).astype(ml_dtypes.bfloat16)  # [B, 128, 4, 512]
    et, ot = _basis_np()
    nc = _get_nc()
    in_maps = [
        {"inp": xt[k * BPC : (k + 1) * BPC], "et": et, "ot": ot}
        for k in range(N_CORES)
    ]
    res = run_bass_kernel_spmd(nc, in_maps, core_ids=list(range(N_CORES)), **spmd_kwargs)
    O = np.empty((B_FULL, 128, 2, 2, 2, H), dtype=np.float32)
    for k in range(N_CORES):
        O[k * BPC : (k + 1) * BPC] = np.asarray(res.results[k]["out"], dtype=np.float32)
    # O[b, u, r, c, ps, q'] = Y_rc[ps*128+u, q'] -> out[b, 2*(ps*128+u)+r, 2*q'+c]
    out = O.transpose(0, 4, 1, 2, 5, 3).reshape(B_FULL, S, S)
    return np.ascontiguousarray(out).reshape(B_FULL, 1, S, S), res


def kernel(img: np.ndarray) -> np.ndarray:
    out, _ = run_sharded(img)
    return out

